# revision 28
# baseline (speedup 1.0000x reference)
"""Trainium2 Bass kernel for nn_Conv2d_StridesAsInput (fractional-stride conv).

Reference semantics: 3x3 conv over bilinearly-resampled patches at positions
pos = out_idx * stride - pad + tap, with stride 2.5, pad 1, dil 1, and
out-of-range taps contributing zero.  Output spatial size uses floor(stride)=2
-> 32x32, so sampling runs past the input and rows/cols >= 26 are bias-only.

Structure exploited (stride == 2.5 exactly):
  * even output rows sample integer x rows (5j + k - 1); odd output rows
    sample half-integer positions -> average of two adjacent rows, same for
    columns.  The 2-tap sums are folded into merged weight variants built on
    device; the 1/2 / 1/4 interpolation scales are applied for free in the
    PSUM->SBUF eviction (activation scale).
  * per parity quadrant (pe, qe) of the output:
        ee: 3x3 taps, weights W,            scale 1
        eo: 3x4 taps, weights merge_l(W),   scale 1/2
        oe: 4x3 taps, weights merge_k(W),   scale 1/2
        oo: 4x4 taps, weights merge_kl(W),  scale 1/4
  * x is shipped zero-padded, phase-major AND bf16:
    xq[b, c, r%5, r//5, c%5, c//5]; each tap's 13x13 grid is a regular
    slice with the contiguous jc dim innermost (fast bf16 moving-operand
    reads).  bf16 matmul runs at 1 col/cycle — same PE rate as fp32r —
    but halves the HBM traffic for x and the weights.

Schedule: per-pr-phase chained input DMAs (first matmul starts after ~20%
of the first image pair has landed), PE pre-warm dummy matmuls during the
DMA lead-in (HAM clock-gate), oh-major quadrant loop with per-image output
stores on the scalar HWDGE ring (input loads keep the sync ring).

Sharding: data-parallel over batch, 4 images per core on 8 cores.
"""

import os

import numpy as np

# ---- problem constants (hardcoded per contract) ----
B, C, H, W = 32, 128, 64, 64
O, KH, KW = 256, 3, 3
OH = OW = 32
PAD = 1
NCORES = 8
BL = B // NCORES   # images per core
NJ = 13            # computed output rows/cols: 0..25; 26..31 are bias-only
RB = 14            # phase-major row/col blocks (70 = 5*14)
STRIDE_VAL = 2.5
NWARM = 70         # PE pre-warm dummy matmuls; bridges the DMA lead-in

_CACHE = {}


def _build_bass():
    import concourse.mybir as mybir
    from concourse import bacc
    from concourse.tile import TileContext
    from concourse.tile_rust import add_dep_helper

    dt = mybir.dt
    mm_dt = dt.bfloat16
    f32 = dt.float32
    AF = mybir.ActivationFunctionType
    ALU = mybir.AluOpType

    nc = bacc.Bacc()
    # blob0 packs the weights and the first x chunk (pair 0, row-phases
    # 0..2) per partition: one DMA, one completion, 16KB descriptors.
    WSZ = KH * KW * O              # 2304
    C0SZ = 2 * 3 * RB * 5 * RB     # pair0 pr0-2, 5880
    blob_in = nc.declare_dram_parameter("blob0", [C, WSZ + C0SZ], mm_dt,
                                        isOutput=False)
    xc1_in = nc.declare_dram_parameter("xc1", [C, 2 * 2 * RB * 5 * RB],
                                       mm_dt, isOutput=False)
    xq1_in = nc.declare_dram_parameter("xq1", [C, 2 * 5 * RB * 5 * RB],
                                       mm_dt, isOutput=False)
    b_in = nc.declare_dram_parameter("bias", [2, 128], f32, isOutput=False)
    out_d = nc.declare_dram_parameter("out", [BL, O, OH, OW], f32, isOutput=True)

    with TileContext(nc) as tc:
        with (
            tc.tile_pool(name="wpool", bufs=1) as wpool,
            tc.tile_pool(name="xpool", bufs=1) as xpool,
            tc.tile_pool(name="opool", bufs=1) as opool,
            tc.tile_pool(name="pspool", bufs=8, space="PSUM") as pspool,
        ):
            bias_sb = wpool.tile([128, 2], f32)
            nc.scalar.dma_start(out=bias_sb,
                                in_=b_in[:].rearrange("h p -> p h"))

            # x input: weights + pair-0 row-phases 0..2 land as one blob
            # (matmuls on pr 0..2 start while pr 3..4 streams), then the
            # pr 3..4 chunk, then pair 1.  Later transfers are chained
            # behind earlier ones: concurrent dma_starts share bandwidth
            # fairly, which would delay the first chunk; each chain link
            # costs ~2us semaphore dead time, so links are few.
            blob = xpool.tile([128, WSZ + C0SZ], mm_dt, name="blob",
                              tag="blob")
            d0 = nc.sync.dma_start(out=blob, in_=blob_in[:])
            xc1 = xpool.tile([128, 2 * 2 * RB * 5 * RB], mm_dt, name="xc1",
                             tag="xc1")
            d1 = nc.sync.dma_start(out=xc1, in_=xc1_in[:])
            add_dep_helper(d1.ins, d0.ins, sync=True,
                           reason="pair0 pr3-4 behind blob")
            xq1 = xpool.tile([128, 2 * 5 * RB * 5 * RB], mm_dt, name="xq1",
                             tag="xq1")
            d2 = nc.sync.dma_start(out=xq1, in_=xq1_in[:])
            add_dep_helper(d2.ins, d1.ins, sync=True,
                           reason="pair1 behind pair0")
            w_dma = d0
            x_last_dma = d2

            w_sb = blob[:, 0:WSZ].rearrange("p (kh kw o) -> p kh kw o",
                                            kh=KH, kw=KW)
            xc0v = blob[:, WSZ:].rearrange(
                "p (b pr jr pc jc) -> p b pr jr pc jc", b=2, pr=3, jr=RB,
                pc=5)
            xc1v = xc1.rearrange(
                "p (b pr jr pc jc) -> p b pr jr pc jc", b=2, pr=2, jr=RB,
                pc=5)
            xq1v = xq1.rearrange(
                "p (b pr jr pc jc) -> p b pr jr pc jc", b=2, pr=5, jr=RB,
                pc=5)

            def x_slice(g, pr, jr, pc, jc):
                if g == 1:
                    return xq1v[:, :, pr, jr : jr + NJ, pc, jc : jc + NJ]
                if pr < 3:
                    return xc0v[:, :, pr, jr : jr + NJ, pc, jc : jc + NJ]
                return xc1v[:, :, pr - 3, jr : jr + NJ, pc, jc : jc + NJ]

            # scratch: zeros for bias-only border fill, dummy warm-up operand
            zt = wpool.tile([128, 2, 26, 32], f32)
            nc.vector.memset(zt, 0.0)
            dm = wpool.tile([128, 128], mm_dt)
            nc.vector.memset(dm, 0.0)

            # PE pre-warm: dummy matmul chain with no data deps; runs during
            # the input-DMA lead-in so the HAM clock gate opens (1.2->2.4GHz)
            # before the first real matmul.
            psd = pspool.tile([128, NJ * NJ * 2], f32, name="ps", tag="ps")
            for i in range(NWARM):
                nc.tensor.matmul(psd[:, 0:128], lhsT=dm, rhs=dm,
                                 start=(i == 0), stop=(i == NWARM - 1))

            # ---- merged tap-sum weight variants (bf16 on DVE) ----
            # merge a length-3 axis into length-4:
            #   v[0]=w[0], v[1]=w[0]+w[1], v[2]=w[1]+w[2], v[3]=w[2]
            def merge3to4(dst, src, axis):
                if axis == 1:
                    nc.vector.tensor_copy(out=dst[:, 0:3], in_=src[:])
                    nc.vector.tensor_copy(out=dst[:, 3:4], in_=src[:, 2:3])
                    nc.vector.tensor_tensor(
                        out=dst[:, 1:3], in0=dst[:, 1:3], in1=src[:, 0:2],
                        op=ALU.add,
                    )
                else:
                    nc.vector.tensor_copy(out=dst[:, :, 0:3], in_=src[:])
                    nc.vector.tensor_copy(out=dst[:, :, 3:4], in_=src[:, :, 2:3])
                    nc.vector.tensor_tensor(
                        out=dst[:, :, 1:3], in0=dst[:, :, 1:3],
                        in1=src[:, :, 0:2], op=ALU.add,
                    )

            # built in quadrant-use order: eo (wl), oe (wk), oo (wkl)
            wl = wpool.tile([128, KH, 4, O], mm_dt)
            wk = wpool.tile([128, 4, KW, O], mm_dt)
            wkl = wpool.tile([128, 4, 4, O], mm_dt)
            merge3to4(wl, w_sb, axis=2)
            merge3to4(wk, w_sb, axis=1)
            merge3to4(wkl, wk, axis=2)

            # quadrant spec: (pe, qe, wtile, n_htaps, n_wtaps, row0, col0,
            # scale); pad-coords: row = row0 + tap_h + 5j, col likewise.
            # Ordered by input-phase arrival: ee+eo need pr 0..2 only.
            quads = [
                (0, 0, w_sb, 3, 3, 0, 0, 1.0),
                (0, 1, wl, 3, 4, 0, 2, 0.5),
                (1, 0, wk, 4, 3, 2, 0, 0.5),
                (1, 1, wkl, 4, 4, 2, 2, 0.25),
            ]

            def do_mm(ps, g, oh, wtile, nh, nw, r0, c0, th, tw):
                rv = r0 + th
                cv = c0 + tw
                pr, jr = rv % 5, rv // 5
                pc, jc = cv % 5, cv // 5
                # taps that sample the zero padding contribute nothing
                # there: rv=0 reads x row -1 at j=0, rv=5 reads x row 64
                # at j=12 (same for columns).  Shrink the access pattern;
                # start=True clears the whole PSUM bank, so skipped cells
                # are plain-written by the first unshrunk tap.
                j0, j1 = (1, NJ) if rv == 0 else (0, NJ - 1) if rv == 5                     else (0, NJ)
                i0, i1 = (1, NJ) if cv == 0 else (0, NJ - 1) if cv == 5                     else (0, NJ)
                rhs = x_slice(g, pr, jr, pc, jc)[:, :, j0:j1, i0:i1]
                out = ps.rearrange("p (b j i) -> p b j i", b=2, j=NJ)[
                    :, :, j0:j1, i0:i1
                ]
                lhsT = wtile[:, th, tw, oh * 128 : (oh + 1) * 128]
                nc.tensor.matmul(
                    out,
                    lhsT=lhsT,
                    rhs=rhs,
                    start=(th == 0 and tw == 0),
                    stop=(th == nh - 1 and tw == nw - 1),
                )

            def evict(ps, ov, oh, pe, qe, qscale):
                # evict computed 26x26 quadrant: out = scale*psum + bias
                nc.scalar.activation(
                    out=ov[:, :, pe : pe + 2 * NJ : 2, qe : qe + 2 * NJ : 2],
                    in_=ps.rearrange("p (b j i) -> p b j i", b=2, j=NJ),
                    func=AF.Identity,
                    scale=qscale,
                    bias=bias_sb[:, oh : oh + 1],
                )

            def border_fill(ov, oh):
                # bias-only border: rows 26..31, and cols 26..31 of rows
                # 0..25 (sampling ran past the input there)
                nc.scalar.activation(
                    out=ov[:, :, 26:32, :], in_=zt[:, :, 0:6, :],
                    func=AF.Identity, scale=1.0,
                    bias=bias_sb[:, oh : oh + 1],
                )
                nc.scalar.activation(
                    out=ov[:, :, 0:26, 26:32], in_=zt[:, :, :, 0:6],
                    func=AF.Identity, scale=1.0,
                    bias=bias_sb[:, oh : oh + 1],
                )

            def store(ot, g, oh, bi, eng=None, after=None):
                # per-image stores, default on the scalar HWDGE ring (keeps
                # the sync ring free for input loads); `after` defers the
                # store so it does not steal HBM bandwidth from input DMAs
                d = (eng or nc.scalar).dma_start(
                    out=out_d[:][
                        2 * g + bi, oh * 128 : (oh + 1) * 128
                    ].rearrange("o h w -> o (h w)"),
                    in_=ot[:, bi],
                )
                if after is not None:
                    add_dep_helper(d.ins, after.ins, sync=True,
                                   reason="defer store behind input loads")

            def run_quad(ps, ov, g, oh, q):
                pe, qe, wtile, nh, nw, r0, c0, qscale = q
                for th in range(nh):
                    for tw in range(nw):
                        do_mm(ps, g, oh, wtile, nh, nw, r0, c0, th, tw)
                evict(ps, ov, oh, pe, qe, qscale)

            def new_ps():
                return pspool.tile([128, NJ * NJ * 2], f32, name="ps",
                                   tag="ps")

            # ---- pair 0: ee+eo (pr 0..2 only) for both output-channel
            # halves first, so matmuls run while the pr 3..4 chunk streams;
            # then oe+oo.  Stores deferred behind the pair-1 input load
            # (HBM bandwidth). ----
            ots0, ovs0 = [], []
            for oh in range(2):
                ot = opool.tile([128, 2, OH * OW], f32, name="ot",
                                tag=f"ot0{oh}")
                ov = ot.rearrange("p b (r q) -> p b r q", r=OH)
                border_fill(ov, oh)
                ots0.append(ot)
                ovs0.append(ov)
            for oh in range(2):
                for q in quads[:2]:
                    run_quad(new_ps(), ovs0[oh], 0, oh, q)
            for oh in range(2):
                for q in quads[2:]:
                    run_quad(new_ps(), ovs0[oh], 0, oh, q)
            for oh in range(2):
                for bi in range(2):
                    store(ots0[oh], 0, oh, bi,
                          eng=(nc.scalar if bi == 0 else nc.sync),
                          after=x_last_dma)

            # ---- pair 1: oh-major (all data resident); the first half's
            # stores overlap the second half's matmuls. ----
            oh = 0
            ot = opool.tile([128, 2, OH * OW], f32, name="ot", tag="ot10")
            ov = ot.rearrange("p b (r q) -> p b r q", r=OH)
            border_fill(ov, oh)
            for q in quads:
                run_quad(new_ps(), ov, 1, oh, q)
            for bi in range(2):
                store(ot, 1, oh, bi, eng=(nc.scalar if bi == 0 else nc.sync))

            # Final half: evictions and border fills split per image so the
            # image-0 store launches while image-1 evictions still run; the
            # last quad chain is the smallest (ee) so its eviction lands
            # early.
            oh = 1
            ot = opool.tile([128, 2, OH * OW], f32, name="ot", tag="ot11")
            ov = ot.rearrange("p b (r q) -> p b r q", r=OH)
            for bi in range(2):
                nc.scalar.activation(
                    out=ov[:, bi, 26:32, :], in_=zt[:, 0, 0:6, :],
                    func=AF.Identity, scale=1.0,
                    bias=bias_sb[:, oh : oh + 1],
                )
                nc.scalar.activation(
                    out=ov[:, bi, 0:26, 26:32], in_=zt[:, 0, :, 0:6],
                    func=AF.Identity, scale=1.0,
                    bias=bias_sb[:, oh : oh + 1],
                )
            for q in quads[::-1]:
                pe, qe, wtile, nh, nw, r0, c0, qscale = q
                ps = new_ps()
                for th in range(nh):
                    for tw in range(nw):
                        do_mm(ps, 1, oh, wtile, nh, nw, r0, c0, th, tw)
                for bi in range(2):
                    nc.scalar.activation(
                        out=ov[:, bi, pe : pe + 2 * NJ : 2,
                               qe : qe + 2 * NJ : 2],
                        in_=ps.rearrange("p (b j i) -> p b j i", b=2,
                                         j=NJ)[:, bi],
                        func=AF.Identity,
                        scale=qscale,
                        bias=bias_sb[:, oh : oh + 1],
                    )
            for bi in range(2):
                store(ot, 1, oh, bi, eng=(nc.scalar if bi == 0 else nc.sync))
    nc.compile()
    return nc


def _host_prep_x(x, np_io):
    """zero-pad to [-1..64+] and shuffle to phase-major blocks:
    [B, C, 5(pr), RB(jr), 5(pc), RB(jc)]."""
    xp = np.zeros((B, C, 5 * RB, 5 * RB), np.float32)
    xp[:, :, 1 : 1 + H, 1 : 1 + W] = x
    xq = np.ascontiguousarray(
        xp.reshape(B, C, RB, 5, RB, 5).transpose(0, 1, 3, 2, 5, 4)
    ).astype(np_io)
    return xq


def _numpy_fallback(x, weight, bias, sh, sw):
    """General fractional-stride conv (the graded stride is always 2.5; this
    covers any other input shape/stride)."""
    Bq, Cq, Hq, Wq = x.shape
    Oq, _, KHq, KWq = weight.shape
    OHq = (Hq + 2 * PAD - (KHq - 1) - 1) // int(np.floor(sh)) + 1
    OWq = (Wq + 2 * PAD - (KWq - 1) - 1) // int(np.floor(sw)) + 1

    def take(arr, p, axis):
        n = arr.shape[axis]
        valid = (p >= 0) & (p < n)
        pc = np.clip(p, 0, n - 1)
        v = np.take(arr, pc.reshape(-1), axis=axis)
        v = v.reshape(arr.shape[:axis] + p.shape + arr.shape[axis + 1 :])
        mask = valid.astype(arr.dtype).reshape(
            (1,) * axis + p.shape + (1,) * (arr.ndim - axis - 1)
        )
        return v * mask

    def bilin(arr, pos, axis):
        p0 = np.floor(pos).astype(np.int64)
        frac = (pos - p0).astype(arr.dtype).reshape(
            (1,) * axis + pos.shape + (1,) * (arr.ndim - axis - 1)
        )
        return take(arr, p0, axis) * (1 - frac) + take(arr, p0 + 1, axis) * frac

    pos_h = (np.arange(OHq, dtype=np.float32)[:, None] * sh
             - PAD + np.arange(KHq, dtype=np.float32)[None, :])
    pos_w = (np.arange(OWq, dtype=np.float32)[:, None] * sw
             - PAD + np.arange(KWq, dtype=np.float32)[None, :])
    rows = bilin(x, pos_h, 2)                      # [B,C,OH,KH,W]
    patches = bilin(rows, pos_w, 4)                # [B,C,OH,KH,OW,KW]
    out = np.einsum("bcpkql,ockl->bopq", patches, weight, optimize=True)
    return (out + bias[None, :, None, None]).astype(np.float32)


def kernel(x, weight, bias, stride_h, stride_w):
    x = np.asarray(x, np.float32)
    weight = np.asarray(weight, np.float32)
    bias = np.asarray(bias, np.float32)
    sh = float(np.asarray(stride_h).reshape(-1)[0])
    sw = float(np.asarray(stride_w).reshape(-1)[0])
    if sh != STRIDE_VAL or sw != STRIDE_VAL or x.shape != (B, C, H, W) \
            or weight.shape != (O, C, KH, KW):
        return _numpy_fallback(x, weight, bias, sh, sw)

    import ml_dtypes

    from concourse.bass_utils import run_bass_kernel_spmd

    if "nc" not in _CACHE:
        _CACHE["nc"] = _build_bass()
    nc = _CACHE["nc"]

    np_io = ml_dtypes.bfloat16
    xq = _host_prep_x(x, np_io)
    wt = weight.transpose(1, 2, 3, 0).reshape(C, -1).astype(np_io)
    bias2 = np.ascontiguousarray(bias.reshape(2, 128))

    in_maps = []
    for i in range(NCORES):
        xc = xq[BL * i : BL * (i + 1)]
        blob0 = np.concatenate(
            [wt, xc[0, :, 0:3].reshape(C, -1), xc[1, :, 0:3].reshape(C, -1)],
            axis=1)
        xc1 = np.concatenate(
            [xc[0, :, 3:5].reshape(C, -1), xc[1, :, 3:5].reshape(C, -1)],
            axis=1)
        xq1 = np.concatenate(
            [xc[2].reshape(C, -1), xc[3].reshape(C, -1)], axis=1)
        in_maps.append({"blob0": np.ascontiguousarray(blob0),
                        "xc1": np.ascontiguousarray(xc1),
                        "xq1": np.ascontiguousarray(xq1),
                        "bias": bias2})
    trace = os.environ.get("CONV_TRACE", "0") == "1"
    res = run_bass_kernel_spmd(nc, in_maps, list(range(NCORES)), trace=trace)
    if trace:
        kernel.last_exec_time_ns = res.exec_time_ns
        kernel.last_results = res
    out = np.concatenate([r["out"] for r in res.results], axis=0)
    return out


# revision 29
# speedup vs baseline: 1.0101x; 1.0101x over previous
"""Trainium2 Bass kernel for nn_Conv2d_StridesAsInput (fractional-stride conv).

Reference semantics: 3x3 conv over bilinearly-resampled patches at positions
pos = out_idx * stride - pad + tap, with stride 2.5, pad 1, dil 1, and
out-of-range taps contributing zero.  Output spatial size uses floor(stride)=2
-> 32x32, so sampling runs past the input and rows/cols >= 26 are bias-only.

Structure exploited (stride == 2.5 exactly):
  * even output rows sample integer x rows (5j + k - 1); odd output rows
    sample half-integer positions -> average of two adjacent rows, same for
    columns.  The 2-tap sums are folded into merged weight variants built on
    device; the 1/2 / 1/4 interpolation scales are applied for free in the
    PSUM->SBUF eviction (activation scale).
  * per parity quadrant (pe, qe) of the output:
        ee: 3x3 taps, weights W,            scale 1
        eo: 3x4 taps, weights merge_l(W),   scale 1/2
        oe: 4x3 taps, weights merge_k(W),   scale 1/2
        oo: 4x4 taps, weights merge_kl(W),  scale 1/4
  * x is shipped zero-padded, phase-major AND bf16:
    xq[b, c, r%5, r//5, c%5, c//5]; each tap's 13x13 grid is a regular
    slice with the contiguous jc dim innermost (fast bf16 moving-operand
    reads).  bf16 matmul runs at 1 col/cycle — same PE rate as fp32r —
    but halves the HBM traffic for x and the weights.

Schedule: per-pr-phase chained input DMAs (first matmul starts after ~20%
of the first image pair has landed), PE pre-warm dummy matmuls during the
DMA lead-in (HAM clock-gate), oh-major quadrant loop with per-image output
stores on the scalar HWDGE ring (input loads keep the sync ring).

Sharding: data-parallel over batch, 4 images per core on 8 cores.
"""

import os

import numpy as np

# ---- problem constants (hardcoded per contract) ----
B, C, H, W = 32, 128, 64, 64
O, KH, KW = 256, 3, 3
OH = OW = 32
PAD = 1
NCORES = 8
BL = B // NCORES   # images per core
NJ = 13            # computed output rows/cols: 0..25; 26..31 are bias-only
RB = 14            # phase-major row/col blocks (70 = 5*14)
STRIDE_VAL = 2.5
NWARM = 75         # PE pre-warm dummy matmuls; bridges the DMA lead-in

_CACHE = {}


def _build_bass():
    import concourse.mybir as mybir
    from concourse import bacc
    from concourse.tile import TileContext
    from concourse.tile_rust import add_dep_helper

    dt = mybir.dt
    mm_dt = dt.bfloat16
    f32 = dt.float32
    AF = mybir.ActivationFunctionType
    ALU = mybir.AluOpType

    nc = bacc.Bacc()
    # blob0 packs the weights and the first x chunk (pair 0, row-phases
    # 0..2) per partition: one DMA, one completion, 16KB descriptors.
    WSZ = KH * KW * O              # 2304
    C0SZ = 2 * 3 * RB * 5 * RB     # pair0 pr0-2, 5880
    blob_in = nc.declare_dram_parameter("blob0", [C, WSZ + C0SZ], mm_dt,
                                        isOutput=False)
    xc1_in = nc.declare_dram_parameter("xc1", [C, 2 * 2 * RB * 5 * RB],
                                       mm_dt, isOutput=False)
    xq1_in = nc.declare_dram_parameter("xq1", [C, 2 * 5 * RB * 5 * RB],
                                       mm_dt, isOutput=False)
    b_in = nc.declare_dram_parameter("bias", [2, 128], f32, isOutput=False)
    out_d = nc.declare_dram_parameter("out", [BL, O, OH, OW], f32, isOutput=True)

    with TileContext(nc) as tc:
        with (
            tc.tile_pool(name="wpool", bufs=1) as wpool,
            tc.tile_pool(name="xpool", bufs=1) as xpool,
            tc.tile_pool(name="opool", bufs=1) as opool,
            tc.tile_pool(name="pspool", bufs=8, space="PSUM") as pspool,
        ):
            bias_sb = wpool.tile([128, 2], f32)
            nc.scalar.dma_start(out=bias_sb,
                                in_=b_in[:].rearrange("h p -> p h"))

            # x input: weights + pair-0 row-phases 0..2 land as one blob
            # (matmuls on pr 0..2 start while pr 3..4 streams), then the
            # pr 3..4 chunk, then pair 1.  Later transfers are chained
            # behind earlier ones: concurrent dma_starts share bandwidth
            # fairly, which would delay the first chunk; each chain link
            # costs ~2us semaphore dead time, so links are few.
            blob = xpool.tile([128, WSZ + C0SZ], mm_dt, name="blob",
                              tag="blob")
            d0 = nc.sync.dma_start(out=blob, in_=blob_in[:])
            xc1 = xpool.tile([128, 2 * 2 * RB * 5 * RB], mm_dt, name="xc1",
                             tag="xc1")
            d1 = nc.sync.dma_start(out=xc1, in_=xc1_in[:])
            add_dep_helper(d1.ins, d0.ins, sync=True,
                           reason="pair0 pr3-4 behind blob")
            xq1 = xpool.tile([128, 2 * 5 * RB * 5 * RB], mm_dt, name="xq1",
                             tag="xq1")
            d2 = nc.sync.dma_start(out=xq1, in_=xq1_in[:])
            add_dep_helper(d2.ins, d1.ins, sync=True,
                           reason="pair1 behind pair0")
            w_dma = d0
            x_last_dma = d2

            w_sb = blob[:, 0:WSZ].rearrange("p (kh kw o) -> p kh kw o",
                                            kh=KH, kw=KW)
            xc0v = blob[:, WSZ:].rearrange(
                "p (b pr jr pc jc) -> p b pr jr pc jc", b=2, pr=3, jr=RB,
                pc=5)
            xc1v = xc1.rearrange(
                "p (b pr jr pc jc) -> p b pr jr pc jc", b=2, pr=2, jr=RB,
                pc=5)
            xq1v = xq1.rearrange(
                "p (b pr jr pc jc) -> p b pr jr pc jc", b=2, pr=5, jr=RB,
                pc=5)

            def x_slice(g, pr, jr, pc, jc):
                if g == 1:
                    return xq1v[:, :, pr, jr : jr + NJ, pc, jc : jc + NJ]
                if pr < 3:
                    return xc0v[:, :, pr, jr : jr + NJ, pc, jc : jc + NJ]
                return xc1v[:, :, pr - 3, jr : jr + NJ, pc, jc : jc + NJ]

            # scratch: zeros for bias-only border fill, dummy warm-up operand
            zt = wpool.tile([128, 2, 26, 32], f32)
            nc.vector.memset(zt, 0.0)
            dm = wpool.tile([128, 128], mm_dt)
            nc.vector.memset(dm, 0.0)

            # PE pre-warm: dummy matmul chain with no data deps; runs during
            # the input-DMA lead-in so the HAM clock gate opens (1.2->2.4GHz)
            # before the first real matmul.
            psd = pspool.tile([128, NJ * NJ * 2], f32, name="ps", tag="ps")
            for i in range(NWARM):
                nc.tensor.matmul(psd[:, 0:128], lhsT=dm, rhs=dm,
                                 start=(i == 0), stop=(i == NWARM - 1))

            # ---- merged tap-sum weight variants (bf16 on DVE) ----
            # merge a length-3 axis into length-4:
            #   v[0]=w[0], v[1]=w[0]+w[1], v[2]=w[1]+w[2], v[3]=w[2]
            def merge3to4(dst, src, axis):
                if axis == 1:
                    nc.vector.tensor_copy(out=dst[:, 0:3], in_=src[:])
                    nc.vector.tensor_copy(out=dst[:, 3:4], in_=src[:, 2:3])
                    nc.vector.tensor_tensor(
                        out=dst[:, 1:3], in0=dst[:, 1:3], in1=src[:, 0:2],
                        op=ALU.add,
                    )
                else:
                    nc.vector.tensor_copy(out=dst[:, :, 0:3], in_=src[:])
                    nc.vector.tensor_copy(out=dst[:, :, 3:4], in_=src[:, :, 2:3])
                    nc.vector.tensor_tensor(
                        out=dst[:, :, 1:3], in0=dst[:, :, 1:3],
                        in1=src[:, :, 0:2], op=ALU.add,
                    )

            # built in quadrant-use order: eo (wl), oe (wk), oo (wkl)
            wl = wpool.tile([128, KH, 4, O], mm_dt)
            wk = wpool.tile([128, 4, KW, O], mm_dt)
            wkl = wpool.tile([128, 4, 4, O], mm_dt)
            merge3to4(wl, w_sb, axis=2)
            merge3to4(wk, w_sb, axis=1)
            merge3to4(wkl, wk, axis=2)

            # quadrant spec: (pe, qe, wtile, n_htaps, n_wtaps, row0, col0,
            # scale); pad-coords: row = row0 + tap_h + 5j, col likewise.
            # Ordered by input-phase arrival: ee+eo need pr 0..2 only.
            quads = [
                (0, 0, w_sb, 3, 3, 0, 0, 1.0),
                (0, 1, wl, 3, 4, 0, 2, 0.5),
                (1, 0, wk, 4, 3, 2, 0, 0.5),
                (1, 1, wkl, 4, 4, 2, 2, 0.25),
            ]

            def do_mm(ps, g, oh, wtile, nh, nw, r0, c0, th, tw):
                rv = r0 + th
                cv = c0 + tw
                pr, jr = rv % 5, rv // 5
                pc, jc = cv % 5, cv // 5
                # taps that sample the zero padding contribute nothing
                # there: rv=0 reads x row -1 at j=0, rv=5 reads x row 64
                # at j=12 (same for columns).  Shrink the access pattern;
                # start=True clears the whole PSUM bank, so skipped cells
                # are plain-written by the first unshrunk tap.
                j0, j1 = (1, NJ) if rv == 0 else (0, NJ - 1) if rv == 5                     else (0, NJ)
                i0, i1 = (1, NJ) if cv == 0 else (0, NJ - 1) if cv == 5                     else (0, NJ)
                rhs = x_slice(g, pr, jr, pc, jc)[:, :, j0:j1, i0:i1]
                out = ps.rearrange("p (b j i) -> p b j i", b=2, j=NJ)[
                    :, :, j0:j1, i0:i1
                ]
                lhsT = wtile[:, th, tw, oh * 128 : (oh + 1) * 128]
                nc.tensor.matmul(
                    out,
                    lhsT=lhsT,
                    rhs=rhs,
                    start=(th == 0 and tw == 0),
                    stop=(th == nh - 1 and tw == nw - 1),
                )

            def evict(ps, ov, oh, pe, qe, qscale):
                # evict computed 26x26 quadrant: out = scale*psum + bias
                nc.scalar.activation(
                    out=ov[:, :, pe : pe + 2 * NJ : 2, qe : qe + 2 * NJ : 2],
                    in_=ps.rearrange("p (b j i) -> p b j i", b=2, j=NJ),
                    func=AF.Identity,
                    scale=qscale,
                    bias=bias_sb[:, oh : oh + 1],
                )

            def border_fill(ov, oh):
                # bias-only border: rows 26..31, and cols 26..31 of rows
                # 0..25 (sampling ran past the input there)
                nc.scalar.activation(
                    out=ov[:, :, 26:32, :], in_=zt[:, :, 0:6, :],
                    func=AF.Identity, scale=1.0,
                    bias=bias_sb[:, oh : oh + 1],
                )
                nc.scalar.activation(
                    out=ov[:, :, 0:26, 26:32], in_=zt[:, :, :, 0:6],
                    func=AF.Identity, scale=1.0,
                    bias=bias_sb[:, oh : oh + 1],
                )

            def store(ot, g, oh, bi, eng=None, after=None):
                # per-image stores, default on the scalar HWDGE ring (keeps
                # the sync ring free for input loads); `after` defers the
                # store so it does not steal HBM bandwidth from input DMAs
                d = (eng or nc.scalar).dma_start(
                    out=out_d[:][
                        2 * g + bi, oh * 128 : (oh + 1) * 128
                    ].rearrange("o h w -> o (h w)"),
                    in_=ot[:, bi],
                )
                if after is not None:
                    add_dep_helper(d.ins, after.ins, sync=True,
                                   reason="defer store behind input loads")

            def run_quad(ps, ov, g, oh, q):
                pe, qe, wtile, nh, nw, r0, c0, qscale = q
                for th in range(nh):
                    for tw in range(nw):
                        do_mm(ps, g, oh, wtile, nh, nw, r0, c0, th, tw)
                evict(ps, ov, oh, pe, qe, qscale)

            def new_ps():
                return pspool.tile([128, NJ * NJ * 2], f32, name="ps",
                                   tag="ps")

            # ---- pair 0: ee+eo (pr 0..2 only) for both output-channel
            # halves first, so matmuls run while the pr 3..4 chunk streams;
            # then oe+oo.  Stores deferred behind the pair-1 input load
            # (HBM bandwidth). ----
            ots0, ovs0 = [], []
            for oh in range(2):
                ot = opool.tile([128, 2, OH * OW], f32, name="ot",
                                tag=f"ot0{oh}")
                ov = ot.rearrange("p b (r q) -> p b r q", r=OH)
                border_fill(ov, oh)
                ots0.append(ot)
                ovs0.append(ov)
            for oh in range(2):
                for q in quads[:2]:
                    run_quad(new_ps(), ovs0[oh], 0, oh, q)
            for oh in range(2):
                for q in quads[2:]:
                    run_quad(new_ps(), ovs0[oh], 0, oh, q)
            for oh in range(2):
                for bi in range(2):
                    store(ots0[oh], 0, oh, bi,
                          eng=(nc.scalar if bi == 0 else nc.sync),
                          after=x_last_dma)

            # ---- pair 1: oh-major (all data resident); the first half's
            # stores overlap the second half's matmuls. ----
            oh = 0
            ot = opool.tile([128, 2, OH * OW], f32, name="ot", tag="ot10")
            ov = ot.rearrange("p b (r q) -> p b r q", r=OH)
            border_fill(ov, oh)
            for q in quads:
                run_quad(new_ps(), ov, 1, oh, q)
            for bi in range(2):
                store(ot, 1, oh, bi, eng=(nc.scalar if bi == 0 else nc.sync))

            # Final half: evictions and border fills split per image so the
            # image-0 store launches while image-1 evictions still run; the
            # last quad chain is the smallest (ee) so its eviction lands
            # early.
            oh = 1
            ot = opool.tile([128, 2, OH * OW], f32, name="ot", tag="ot11")
            ov = ot.rearrange("p b (r q) -> p b r q", r=OH)
            for bi in range(2):
                nc.scalar.activation(
                    out=ov[:, bi, 26:32, :], in_=zt[:, 0, 0:6, :],
                    func=AF.Identity, scale=1.0,
                    bias=bias_sb[:, oh : oh + 1],
                )
                nc.scalar.activation(
                    out=ov[:, bi, 0:26, 26:32], in_=zt[:, 0, :, 0:6],
                    func=AF.Identity, scale=1.0,
                    bias=bias_sb[:, oh : oh + 1],
                )
            for q in quads[::-1]:
                pe, qe, wtile, nh, nw, r0, c0, qscale = q
                ps = new_ps()
                for th in range(nh):
                    for tw in range(nw):
                        do_mm(ps, 1, oh, wtile, nh, nw, r0, c0, th, tw)
                for bi in range(2):
                    nc.scalar.activation(
                        out=ov[:, bi, pe : pe + 2 * NJ : 2,
                               qe : qe + 2 * NJ : 2],
                        in_=ps.rearrange("p (b j i) -> p b j i", b=2,
                                         j=NJ)[:, bi],
                        func=AF.Identity,
                        scale=qscale,
                        bias=bias_sb[:, oh : oh + 1],
                    )
            for bi in range(2):
                store(ot, 1, oh, bi, eng=(nc.scalar if bi == 0 else nc.sync))
    nc.compile()
    return nc


def _host_prep_x(x, np_io):
    """zero-pad to [-1..64+] and shuffle to phase-major blocks:
    [B, C, 5(pr), RB(jr), 5(pc), RB(jc)]."""
    xp = np.zeros((B, C, 5 * RB, 5 * RB), np.float32)
    xp[:, :, 1 : 1 + H, 1 : 1 + W] = x
    xq = np.ascontiguousarray(
        xp.reshape(B, C, RB, 5, RB, 5).transpose(0, 1, 3, 2, 5, 4)
    ).astype(np_io)
    return xq


def _numpy_fallback(x, weight, bias, sh, sw):
    """General fractional-stride conv (the graded stride is always 2.5; this
    covers any other input shape/stride)."""
    Bq, Cq, Hq, Wq = x.shape
    Oq, _, KHq, KWq = weight.shape
    OHq = (Hq + 2 * PAD - (KHq - 1) - 1) // int(np.floor(sh)) + 1
    OWq = (Wq + 2 * PAD - (KWq - 1) - 1) // int(np.floor(sw)) + 1

    def take(arr, p, axis):
        n = arr.shape[axis]
        valid = (p >= 0) & (p < n)
        pc = np.clip(p, 0, n - 1)
        v = np.take(arr, pc.reshape(-1), axis=axis)
        v = v.reshape(arr.shape[:axis] + p.shape + arr.shape[axis + 1 :])
        mask = valid.astype(arr.dtype).reshape(
            (1,) * axis + p.shape + (1,) * (arr.ndim - axis - 1)
        )
        return v * mask

    def bilin(arr, pos, axis):
        p0 = np.floor(pos).astype(np.int64)
        frac = (pos - p0).astype(arr.dtype).reshape(
            (1,) * axis + pos.shape + (1,) * (arr.ndim - axis - 1)
        )
        return take(arr, p0, axis) * (1 - frac) + take(arr, p0 + 1, axis) * frac

    pos_h = (np.arange(OHq, dtype=np.float32)[:, None] * sh
             - PAD + np.arange(KHq, dtype=np.float32)[None, :])
    pos_w = (np.arange(OWq, dtype=np.float32)[:, None] * sw
             - PAD + np.arange(KWq, dtype=np.float32)[None, :])
    rows = bilin(x, pos_h, 2)                      # [B,C,OH,KH,W]
    patches = bilin(rows, pos_w, 4)                # [B,C,OH,KH,OW,KW]
    out = np.einsum("bcpkql,ockl->bopq", patches, weight, optimize=True)
    return (out + bias[None, :, None, None]).astype(np.float32)


def kernel(x, weight, bias, stride_h, stride_w):
    x = np.asarray(x, np.float32)
    weight = np.asarray(weight, np.float32)
    bias = np.asarray(bias, np.float32)
    sh = float(np.asarray(stride_h).reshape(-1)[0])
    sw = float(np.asarray(stride_w).reshape(-1)[0])
    if sh != STRIDE_VAL or sw != STRIDE_VAL or x.shape != (B, C, H, W) \
            or weight.shape != (O, C, KH, KW):
        return _numpy_fallback(x, weight, bias, sh, sw)

    import ml_dtypes

    from concourse.bass_utils import run_bass_kernel_spmd

    if "nc" not in _CACHE:
        _CACHE["nc"] = _build_bass()
    nc = _CACHE["nc"]

    np_io = ml_dtypes.bfloat16
    xq = _host_prep_x(x, np_io)
    wt = weight.transpose(1, 2, 3, 0).reshape(C, -1).astype(np_io)
    bias2 = np.ascontiguousarray(bias.reshape(2, 128))

    in_maps = []
    for i in range(NCORES):
        xc = xq[BL * i : BL * (i + 1)]
        blob0 = np.concatenate(
            [wt, xc[0, :, 0:3].reshape(C, -1), xc[1, :, 0:3].reshape(C, -1)],
            axis=1)
        xc1 = np.concatenate(
            [xc[0, :, 3:5].reshape(C, -1), xc[1, :, 3:5].reshape(C, -1)],
            axis=1)
        xq1 = np.concatenate(
            [xc[2].reshape(C, -1), xc[3].reshape(C, -1)], axis=1)
        in_maps.append({"blob0": np.ascontiguousarray(blob0),
                        "xc1": np.ascontiguousarray(xc1),
                        "xq1": np.ascontiguousarray(xq1),
                        "bias": bias2})
    trace = os.environ.get("CONV_TRACE", "0") == "1"
    res = run_bass_kernel_spmd(nc, in_maps, list(range(NCORES)), trace=trace)
    if trace:
        kernel.last_exec_time_ns = res.exec_time_ns
        kernel.last_results = res
    out = np.concatenate([r["out"] for r in res.results], axis=0)
    return out


# revision 30
# speedup vs baseline: 1.0331x; 1.0228x over previous
"""Trainium2 Bass kernel for nn_Conv2d_StridesAsInput (fractional-stride conv).

Reference semantics: 3x3 conv over bilinearly-resampled patches at positions
pos = out_idx * stride - pad + tap, with stride 2.5, pad 1, dil 1, and
out-of-range taps contributing zero.  Output spatial size uses floor(stride)=2
-> 32x32, so sampling runs past the input and rows/cols >= 26 are bias-only.

Structure exploited (stride == 2.5 exactly):
  * even output rows sample integer x rows (5j + k - 1); odd output rows
    sample half-integer positions -> average of two adjacent rows, same for
    columns.  The 2-tap sums are folded into merged weight variants built on
    device; the 1/2 / 1/4 interpolation scales are applied for free in the
    PSUM->SBUF eviction (activation scale).
  * per parity quadrant (pe, qe) of the output:
        ee: 3x3 taps, weights W,            scale 1
        eo: 3x4 taps, weights merge_l(W),   scale 1/2
        oe: 4x3 taps, weights merge_k(W),   scale 1/2
        oo: 4x4 taps, weights merge_kl(W),  scale 1/4
  * x is shipped zero-padded, phase-major AND bf16:
    xq[b, c, r%5, r//5, c%5, c//5]; each tap's 13x13 grid is a regular
    slice with the contiguous jc dim innermost (fast bf16 moving-operand
    reads).  bf16 matmul runs at 1 col/cycle — same PE rate as fp32r —
    but halves the HBM traffic for x and the weights.

Schedule: per-pr-phase chained input DMAs (first matmul starts after ~20%
of the first image pair has landed), PE pre-warm dummy matmuls during the
DMA lead-in (HAM clock-gate), oh-major quadrant loop with per-image output
stores on the scalar HWDGE ring (input loads keep the sync ring).

Sharding: data-parallel over batch, 4 images per core on 8 cores.
"""

import os

import numpy as np

# ---- problem constants (hardcoded per contract) ----
B, C, H, W = 32, 128, 64, 64
O, KH, KW = 256, 3, 3
OH = OW = 32
PAD = 1
NCORES = 8
BL = B // NCORES   # images per core
NJ = 13            # computed output rows/cols: 0..25; 26..31 are bias-only
RB = 14            # phase-major row/col blocks (70 = 5*14)
STRIDE_VAL = 2.5
NWARM = 50         # PE pre-warm dummy matmuls: ends ~1.5-2us before
                   # the first x chunk lands (HAM stays warm for idle
                   # gaps < 3.4us; overrunning delays real matmuls)

_CACHE = {}


def _build_bass():
    import concourse.mybir as mybir
    from concourse import bacc
    from concourse.tile import TileContext
    from concourse.tile_rust import add_dep_helper

    dt = mybir.dt
    mm_dt = dt.bfloat16
    f32 = dt.float32
    AF = mybir.ActivationFunctionType
    ALU = mybir.AluOpType

    nc = bacc.Bacc()
    # blob0 packs the weights and the first x chunk (pair 0, row-phases
    # 0..2) per partition: one DMA, one completion, 16KB descriptors.
    WSZ = KH * KW * O              # 2304
    C0SZ = 2 * 3 * RB * 5 * RB     # pair0 pr0-2, 5880
    blob_in = nc.declare_dram_parameter("blob0", [C, WSZ + C0SZ], mm_dt,
                                        isOutput=False)
    xc1_in = nc.declare_dram_parameter("xc1", [C, 2 * 2 * RB * 5 * RB],
                                       mm_dt, isOutput=False)
    xq1_in = nc.declare_dram_parameter("xq1", [C, 2 * 5 * RB * 5 * RB],
                                       mm_dt, isOutput=False)
    b_in = nc.declare_dram_parameter("bias", [2, 128], f32, isOutput=False)
    out_d = nc.declare_dram_parameter("out", [BL, O, OH, OW], f32, isOutput=True)

    with TileContext(nc) as tc:
        with (
            tc.tile_pool(name="wpool", bufs=1) as wpool,
            tc.tile_pool(name="xpool", bufs=1) as xpool,
            tc.tile_pool(name="opool", bufs=1) as opool,
            tc.tile_pool(name="pspool", bufs=8, space="PSUM") as pspool,
        ):
            bias_sb = wpool.tile([128, 2], f32)
            nc.scalar.dma_start(out=bias_sb,
                                in_=b_in[:].rearrange("h p -> p h"))

            # x input: weights + pair-0 row-phases 0..2 land as one blob
            # (matmuls on pr 0..2 start while pr 3..4 streams), then the
            # pr 3..4 chunk, then pair 1.  Later transfers are chained
            # behind earlier ones: concurrent dma_starts share bandwidth
            # fairly, which would delay the first chunk; each chain link
            # costs ~2us semaphore dead time, so links are few.
            blob = xpool.tile([128, WSZ + C0SZ], mm_dt, name="blob",
                              tag="blob")
            d0 = nc.sync.dma_start(out=blob, in_=blob_in[:])
            xc1 = xpool.tile([128, 2 * 2 * RB * 5 * RB], mm_dt, name="xc1",
                             tag="xc1")
            d1 = nc.sync.dma_start(out=xc1, in_=xc1_in[:])
            add_dep_helper(d1.ins, d0.ins, sync=True,
                           reason="pair0 pr3-4 behind blob")
            xq1 = xpool.tile([128, 2 * 5 * RB * 5 * RB], mm_dt, name="xq1",
                             tag="xq1")
            d2 = nc.sync.dma_start(out=xq1, in_=xq1_in[:])
            add_dep_helper(d2.ins, d1.ins, sync=True,
                           reason="pair1 behind pair0")
            w_dma = d0
            x_last_dma = d2

            w_sb = blob[:, 0:WSZ].rearrange("p (kh kw o) -> p kh kw o",
                                            kh=KH, kw=KW)
            xc0v = blob[:, WSZ:].rearrange(
                "p (b pr jr pc jc) -> p b pr jr pc jc", b=2, pr=3, jr=RB,
                pc=5)
            xc1v = xc1.rearrange(
                "p (b pr jr pc jc) -> p b pr jr pc jc", b=2, pr=2, jr=RB,
                pc=5)
            xq1v = xq1.rearrange(
                "p (b pr jr pc jc) -> p b pr jr pc jc", b=2, pr=5, jr=RB,
                pc=5)

            def x_slice(g, pr, jr, pc, jc):
                if g == 1:
                    return xq1v[:, :, pr, jr : jr + NJ, pc, jc : jc + NJ]
                if pr < 3:
                    return xc0v[:, :, pr, jr : jr + NJ, pc, jc : jc + NJ]
                return xc1v[:, :, pr - 3, jr : jr + NJ, pc, jc : jc + NJ]

            # scratch: zeros for bias-only border fill, dummy warm-up operand
            zt = wpool.tile([128, 2, 26, 32], f32)
            nc.vector.memset(zt, 0.0)
            dm = wpool.tile([128, 128], mm_dt)
            nc.vector.memset(dm, 0.0)

            # PE pre-warm: dummy matmul chain with no data deps; runs during
            # the input-DMA lead-in so the HAM clock gate opens (1.2->2.4GHz)
            # before the first real matmul.
            psd = pspool.tile([128, NJ * NJ * 2], f32, name="ps", tag="ps")
            for i in range(NWARM):
                nc.tensor.matmul(psd[:, 0:128], lhsT=dm, rhs=dm,
                                 start=(i == 0), stop=(i == NWARM - 1))

            # ---- merged tap-sum weight variants (bf16 on DVE) ----
            # merge a length-3 axis into length-4:
            #   v[0]=w[0], v[1]=w[0]+w[1], v[2]=w[1]+w[2], v[3]=w[2]
            def merge3to4(dst, src, axis):
                if axis == 1:
                    nc.vector.tensor_copy(out=dst[:, 0:3], in_=src[:])
                    nc.vector.tensor_copy(out=dst[:, 3:4], in_=src[:, 2:3])
                    nc.vector.tensor_tensor(
                        out=dst[:, 1:3], in0=dst[:, 1:3], in1=src[:, 0:2],
                        op=ALU.add,
                    )
                else:
                    nc.vector.tensor_copy(out=dst[:, :, 0:3], in_=src[:])
                    nc.vector.tensor_copy(out=dst[:, :, 3:4], in_=src[:, :, 2:3])
                    nc.vector.tensor_tensor(
                        out=dst[:, :, 1:3], in0=dst[:, :, 1:3],
                        in1=src[:, :, 0:2], op=ALU.add,
                    )

            # built in quadrant-use order: eo (wl), oe (wk), oo (wkl)
            wl = wpool.tile([128, KH, 4, O], mm_dt)
            wk = wpool.tile([128, 4, KW, O], mm_dt)
            wkl = wpool.tile([128, 4, 4, O], mm_dt)
            merge3to4(wl, w_sb, axis=2)
            merge3to4(wk, w_sb, axis=1)
            merge3to4(wkl, wk, axis=2)

            # quadrant spec: (pe, qe, wtile, n_htaps, n_wtaps, row0, col0,
            # scale); pad-coords: row = row0 + tap_h + 5j, col likewise.
            # Ordered by input-phase arrival: ee+eo need pr 0..2 only.
            quads = [
                (0, 0, w_sb, 3, 3, 0, 0, 1.0),
                (0, 1, wl, 3, 4, 0, 2, 0.5),
                (1, 0, wk, 4, 3, 2, 0, 0.5),
                (1, 1, wkl, 4, 4, 2, 2, 0.25),
            ]

            def do_mm(ps, g, oh, wtile, nh, nw, r0, c0, th, tw):
                rv = r0 + th
                cv = c0 + tw
                pr, jr = rv % 5, rv // 5
                pc, jc = cv % 5, cv // 5
                # taps that sample the zero padding contribute nothing
                # there: rv=0 reads x row -1 at j=0, rv=5 reads x row 64
                # at j=12 (same for columns).  Shrink the access pattern;
                # start=True clears the whole PSUM bank, so skipped cells
                # are plain-written by the first unshrunk tap.
                j0, j1 = (1, NJ) if rv == 0 else (0, NJ - 1) if rv == 5                     else (0, NJ)
                i0, i1 = (1, NJ) if cv == 0 else (0, NJ - 1) if cv == 5                     else (0, NJ)
                rhs = x_slice(g, pr, jr, pc, jc)[:, :, j0:j1, i0:i1]
                out = ps.rearrange("p (b j i) -> p b j i", b=2, j=NJ)[
                    :, :, j0:j1, i0:i1
                ]
                lhsT = wtile[:, th, tw, oh * 128 : (oh + 1) * 128]
                nc.tensor.matmul(
                    out,
                    lhsT=lhsT,
                    rhs=rhs,
                    start=(th == 0 and tw == 0),
                    stop=(th == nh - 1 and tw == nw - 1),
                )

            def evict(ps, ov, oh, pe, qe, qscale):
                # evict computed 26x26 quadrant: out = scale*psum + bias
                nc.scalar.activation(
                    out=ov[:, :, pe : pe + 2 * NJ : 2, qe : qe + 2 * NJ : 2],
                    in_=ps.rearrange("p (b j i) -> p b j i", b=2, j=NJ),
                    func=AF.Identity,
                    scale=qscale,
                    bias=bias_sb[:, oh : oh + 1],
                )

            def border_fill(ov, oh):
                # bias-only border: rows 26..31, and cols 26..31 of rows
                # 0..25 (sampling ran past the input there)
                nc.scalar.activation(
                    out=ov[:, :, 26:32, :], in_=zt[:, :, 0:6, :],
                    func=AF.Identity, scale=1.0,
                    bias=bias_sb[:, oh : oh + 1],
                )
                nc.scalar.activation(
                    out=ov[:, :, 0:26, 26:32], in_=zt[:, :, :, 0:6],
                    func=AF.Identity, scale=1.0,
                    bias=bias_sb[:, oh : oh + 1],
                )

            def store(ot, g, oh, bi, eng=None, after=None):
                # per-image stores, default on the scalar HWDGE ring (keeps
                # the sync ring free for input loads); `after` defers the
                # store so it does not steal HBM bandwidth from input DMAs
                d = (eng or nc.scalar).dma_start(
                    out=out_d[:][
                        2 * g + bi, oh * 128 : (oh + 1) * 128
                    ].rearrange("o h w -> o (h w)"),
                    in_=ot[:, bi],
                )
                if after is not None:
                    add_dep_helper(d.ins, after.ins, sync=True,
                                   reason="defer store behind input loads")

            def run_quad(ps, ov, g, oh, q):
                pe, qe, wtile, nh, nw, r0, c0, qscale = q
                for th in range(nh):
                    for tw in range(nw):
                        do_mm(ps, g, oh, wtile, nh, nw, r0, c0, th, tw)
                evict(ps, ov, oh, pe, qe, qscale)

            def new_ps():
                return pspool.tile([128, NJ * NJ * 2], f32, name="ps",
                                   tag="ps")

            # ---- pair 0: ee+eo (pr 0..2 only) for both output-channel
            # halves first, so matmuls run while the pr 3..4 chunk streams;
            # then oe+oo.  Stores deferred behind the pair-1 input load
            # (HBM bandwidth). ----
            ots0, ovs0 = [], []
            for oh in range(2):
                ot = opool.tile([128, 2, OH * OW], f32, name="ot",
                                tag=f"ot0{oh}")
                ov = ot.rearrange("p b (r q) -> p b r q", r=OH)
                border_fill(ov, oh)
                ots0.append(ot)
                ovs0.append(ov)
            for oh in range(2):
                for q in quads[:2]:
                    run_quad(new_ps(), ovs0[oh], 0, oh, q)
            for oh in range(2):
                for q in quads[2:]:
                    run_quad(new_ps(), ovs0[oh], 0, oh, q)
            for oh in range(2):
                for bi in range(2):
                    store(ots0[oh], 0, oh, bi,
                          eng=(nc.scalar if bi == 0 else nc.sync),
                          after=x_last_dma)

            # ---- pair 1: oh-major (all data resident); the first half's
            # stores overlap the second half's matmuls. ----
            oh = 0
            ot = opool.tile([128, 2, OH * OW], f32, name="ot", tag="ot10")
            ov = ot.rearrange("p b (r q) -> p b r q", r=OH)
            border_fill(ov, oh)
            for q in quads:
                run_quad(new_ps(), ov, 1, oh, q)
            for bi in range(2):
                store(ot, 1, oh, bi, eng=(nc.scalar if bi == 0 else nc.sync))

            # Final half: evictions and border fills split per image so the
            # image-0 store launches while image-1 evictions still run; the
            # last quad chain is the smallest (ee) so its eviction lands
            # early.
            oh = 1
            ot = opool.tile([128, 2, OH * OW], f32, name="ot", tag="ot11")
            ov = ot.rearrange("p b (r q) -> p b r q", r=OH)
            for bi in range(2):
                nc.scalar.activation(
                    out=ov[:, bi, 26:32, :], in_=zt[:, 0, 0:6, :],
                    func=AF.Identity, scale=1.0,
                    bias=bias_sb[:, oh : oh + 1],
                )
                nc.scalar.activation(
                    out=ov[:, bi, 0:26, 26:32], in_=zt[:, 0, :, 0:6],
                    func=AF.Identity, scale=1.0,
                    bias=bias_sb[:, oh : oh + 1],
                )
            for q in quads[::-1]:
                pe, qe, wtile, nh, nw, r0, c0, qscale = q
                ps = new_ps()
                for th in range(nh):
                    for tw in range(nw):
                        do_mm(ps, 1, oh, wtile, nh, nw, r0, c0, th, tw)
                for bi in range(2):
                    nc.scalar.activation(
                        out=ov[:, bi, pe : pe + 2 * NJ : 2,
                               qe : qe + 2 * NJ : 2],
                        in_=ps.rearrange("p (b j i) -> p b j i", b=2,
                                         j=NJ)[:, bi],
                        func=AF.Identity,
                        scale=qscale,
                        bias=bias_sb[:, oh : oh + 1],
                    )
            for bi in range(2):
                store(ot, 1, oh, bi, eng=(nc.scalar if bi == 0 else nc.sync))
    nc.compile()
    return nc


def _host_prep_x(x, np_io):
    """zero-pad to [-1..64+] and shuffle to phase-major blocks:
    [B, C, 5(pr), RB(jr), 5(pc), RB(jc)]."""
    xp = np.zeros((B, C, 5 * RB, 5 * RB), np.float32)
    xp[:, :, 1 : 1 + H, 1 : 1 + W] = x
    xq = np.ascontiguousarray(
        xp.reshape(B, C, RB, 5, RB, 5).transpose(0, 1, 3, 2, 5, 4)
    ).astype(np_io)
    return xq


def _numpy_fallback(x, weight, bias, sh, sw):
    """General fractional-stride conv (the graded stride is always 2.5; this
    covers any other input shape/stride)."""
    Bq, Cq, Hq, Wq = x.shape
    Oq, _, KHq, KWq = weight.shape
    OHq = (Hq + 2 * PAD - (KHq - 1) - 1) // int(np.floor(sh)) + 1
    OWq = (Wq + 2 * PAD - (KWq - 1) - 1) // int(np.floor(sw)) + 1

    def take(arr, p, axis):
        n = arr.shape[axis]
        valid = (p >= 0) & (p < n)
        pc = np.clip(p, 0, n - 1)
        v = np.take(arr, pc.reshape(-1), axis=axis)
        v = v.reshape(arr.shape[:axis] + p.shape + arr.shape[axis + 1 :])
        mask = valid.astype(arr.dtype).reshape(
            (1,) * axis + p.shape + (1,) * (arr.ndim - axis - 1)
        )
        return v * mask

    def bilin(arr, pos, axis):
        p0 = np.floor(pos).astype(np.int64)
        frac = (pos - p0).astype(arr.dtype).reshape(
            (1,) * axis + pos.shape + (1,) * (arr.ndim - axis - 1)
        )
        return take(arr, p0, axis) * (1 - frac) + take(arr, p0 + 1, axis) * frac

    pos_h = (np.arange(OHq, dtype=np.float32)[:, None] * sh
             - PAD + np.arange(KHq, dtype=np.float32)[None, :])
    pos_w = (np.arange(OWq, dtype=np.float32)[:, None] * sw
             - PAD + np.arange(KWq, dtype=np.float32)[None, :])
    rows = bilin(x, pos_h, 2)                      # [B,C,OH,KH,W]
    patches = bilin(rows, pos_w, 4)                # [B,C,OH,KH,OW,KW]
    out = np.einsum("bcpkql,ockl->bopq", patches, weight, optimize=True)
    return (out + bias[None, :, None, None]).astype(np.float32)


def kernel(x, weight, bias, stride_h, stride_w):
    x = np.asarray(x, np.float32)
    weight = np.asarray(weight, np.float32)
    bias = np.asarray(bias, np.float32)
    sh = float(np.asarray(stride_h).reshape(-1)[0])
    sw = float(np.asarray(stride_w).reshape(-1)[0])
    if sh != STRIDE_VAL or sw != STRIDE_VAL or x.shape != (B, C, H, W) \
            or weight.shape != (O, C, KH, KW):
        return _numpy_fallback(x, weight, bias, sh, sw)

    import ml_dtypes

    from concourse.bass_utils import run_bass_kernel_spmd

    if "nc" not in _CACHE:
        _CACHE["nc"] = _build_bass()
    nc = _CACHE["nc"]

    np_io = ml_dtypes.bfloat16
    xq = _host_prep_x(x, np_io)
    wt = weight.transpose(1, 2, 3, 0).reshape(C, -1).astype(np_io)
    bias2 = np.ascontiguousarray(bias.reshape(2, 128))

    in_maps = []
    for i in range(NCORES):
        xc = xq[BL * i : BL * (i + 1)]
        blob0 = np.concatenate(
            [wt, xc[0, :, 0:3].reshape(C, -1), xc[1, :, 0:3].reshape(C, -1)],
            axis=1)
        xc1 = np.concatenate(
            [xc[0, :, 3:5].reshape(C, -1), xc[1, :, 3:5].reshape(C, -1)],
            axis=1)
        xq1 = np.concatenate(
            [xc[2].reshape(C, -1), xc[3].reshape(C, -1)], axis=1)
        in_maps.append({"blob0": np.ascontiguousarray(blob0),
                        "xc1": np.ascontiguousarray(xc1),
                        "xq1": np.ascontiguousarray(xq1),
                        "bias": bias2})
    trace = os.environ.get("CONV_TRACE", "0") == "1"
    res = run_bass_kernel_spmd(nc, in_maps, list(range(NCORES)), trace=trace)
    if trace:
        kernel.last_exec_time_ns = res.exec_time_ns
        kernel.last_results = res
    out = np.concatenate([r["out"] for r in res.results], axis=0)
    return out
